# revision 30
# baseline (speedup 1.0000x reference)
"""Trainium2 Bass kernel for nn_Attention (B=8, N=1024, C=768, 12 heads).

Sharding: pure data-parallel over batch — 8 cores, one batch element per
core, full weights replicated to every core. No collectives.

Per-core design: the Scalar engine's 96 softmax-exp activations
(12 heads x 8 key tiles x [128, 1024]) are the hard floor (~107 us), so
the kernel keeps ACT saturated with exp while tensor-engine work hides
underneath it:

  prologue: inputs land as a few large strided DMAs in priority order
            (pair-0 q/k weights and x from the otherwise-idle Scalar
            queue, the rest from Sync); PE warm-up burst during the
            wait.
  per head-pair hp (heads 2hp, 2hp+1 share qkT tiles in partitions
  0-63 / 64-127):
    qkT proj: qT/kT = w_qkv_tile.T @ xT feature-major, DVE-drained to
              bf16.
    mk loop:  S^T[tk,tq] = kT_h.T @ qT_h (K=64; the two heads target
              row groups 0-1 / 2-3), exp on ACT -> et tiles [128,1024]
              bf16 in SBUF. Emitted via high_priority so exp is never
              starved.
    PV:       O^T_unnorm[65,tq] = v_aug.T @ attn_exp accumulated over
              mk from staged et tiles, scheduled into the ACT-bound
              window of pair hp+1. The v_aug ones column makes row 64
              the softmax denominator for free.
    norm:     K=1 ones-matmul broadcasts the denominator row across 64
              partitions, DVE reciprocal + multiply.
  v-proj fills early PE slack; the final projection (out = oT.T @
  w_proj + bias, bias added in the DVE drain) runs at the tail with
  output DMAs alternating Sync/GpSimd queues.

All matmuls run in bf16 (fp8 was tried and rejected: e4m3's ~3.6% RMS
quantization propagates to ~4-7e-2 absmax-relative output error, past
the 2e-2 gate).
"""

import os
import sys

import numpy as np

for _p in ("/opt/trn_rl_repo",):
    if os.path.isdir(_p) and _p not in sys.path:
        sys.path.append(_p)

import ml_dtypes

import concourse.bacc as bacc
import concourse.mybir as mybir
import concourse.tile as tile
from concourse.bass_utils import run_bass_kernel_spmd

F32 = mybir.dt.float32
BF16 = mybir.dt.bfloat16
F8 = mybir.dt.float8e4
EXP = mybir.ActivationFunctionType.Exp
DR = mybir.MatmulPerfMode.DoubleRow
MULT = mybir.AluOpType.mult
ADD = mybir.AluOpType.add

P = 128
B, N, C = 8, 1024, 768
NH, HD = 12, 64
C3 = 3 * C
KC = C // P          # 6 contraction tiles over channels
KJ = KC // 2         # 3 DoubleRow contraction pairs
NT = N // P          # 8 token tiles of 128
NT2 = N // 512       # 2 token slices of 512
NHALF = NH // 2      # 6 head pairs
VA = HD + 1          # 65: head dim + ones column
VP = P               # padded per-head stride in v_aug
SCALE = float(HD) ** -0.5
WSCALE = 16.0        # host-side fp8 weight prescale
WINV = 1.0 / WSCALE
ET_BUFS = 28         # staged exp tiles (1.75 pairs worth)

_CACHE = {}


def _emit(nc, tc):
    xT_d = nc.dram_tensor("xT", [C, N], BF16, kind="ExternalInput")
    wqkv_d = nc.dram_tensor("w_qkv", [C, C3], BF16, kind="ExternalInput")
    wproj_d = nc.dram_tensor("w_proj", [C, C], BF16, kind="ExternalInput")
    bproj_d = nc.dram_tensor("b_proj", [1, C], BF16, kind="ExternalInput")
    out_d = nc.dram_tensor("out", [N, C], F32, kind="ExternalOutput")

    mm = nc.tensor.matmul

    from contextlib import ExitStack

    with ExitStack() as es:
        const = es.enter_context(tc.tile_pool(name="const", bufs=1))
        big = es.enter_context(tc.tile_pool(name="big", bufs=1))
        etp = es.enter_context(tc.tile_pool(name="etp", bufs=ET_BUFS))
        small = es.enter_context(tc.tile_pool(name="small", bufs=2))

        onesc = const.tile([P, NH], F32, tag="onesc", name="onesc")
        nc.vector.memset(onesc[:], 1.0)
        onesf = const.tile([1, P], F32, tag="onesf", name="onesf")
        nc.vector.memset(onesf[:], 1.0)
        ones = const.tile([1, P], BF16, tag="ones", name="ones")
        nc.vector.tensor_copy(ones[:], onesf[:])
        bproj_sb = const.tile([1, C], BF16, tag="bproj", name="bproj")
        nc.sync.dma_start(bproj_sb[:], bproj_d.ap())
        bias_bc = const.tile([P, C], F32, tag="bias_bc", name="bias_bc")

        x8_all = big.tile([P, KC * N], BF16, tag="x8", name="x8")
        wq_all = big.tile([P, KC * C3], BF16, tag="wq", name="wq")
        wp_all = big.tile([P, KC * C], BF16, tag="wp", name="wp")
        qkT = [big.tile([P, N], BF16, tag=f"qk{m}", name=f"qk{m}")
               for m in range(2 * NHALF)]
        vaug = [big.tile([P, NH * VP], BF16, tag=f"va{t}", name=f"va{t}")
                for t in range(NT)]
        oT = [big.tile([P, N], BF16, tag=f"oT{i}", name=f"oT{i}")
              for i in range(KC)]

        def xT(k):
            return x8_all[:, k * N:(k + 1) * N]

        def wq(k):
            return wq_all[:, k * C3:(k + 1) * C3]

        def wp(k):
            return wp_all[:, k * C:(k + 1) * C]

        # Input DMAs as a few large strided transfers in priority order
        # (pair-0 q/k weights and x from the otherwise-idle Scalar queue,
        # the rest from Sync).
        wqkv_src = wqkv_d.ap().rearrange("(k p) c -> p k c", p=P)
        wq_dst = wq_all[:].rearrange("p (k c) -> p k c", c=C3)
        x_src = xT_d.ap().rearrange("(k p) n -> p k n", p=P)
        x_dst = x8_all[:].rearrange("p (k n) -> p k n", n=N)
        # pair-0 q and k weight slices
        nc.scalar.dma_start(wq_dst[:, :, 0:P], wqkv_src[:, :, 0:P])
        nc.scalar.dma_start(wq_dst[:, :, C:C + P], wqkv_src[:, :, C:C + P])
        nc.scalar.dma_start(x_dst[:, 0:3, :], x_src[:, 0:3, :])
        nc.scalar.dma_start(x_dst[:, 3:KC, :], x_src[:, 3:KC, :])
        # v weights
        nc.sync.dma_start(wq_dst[:, :, 2 * C:C3], wqkv_src[:, :, 2 * C:C3])
        # q and k weights for pairs 1-5
        nc.sync.dma_start(wq_dst[:, :, P:C], wqkv_src[:, :, P:C])
        nc.sync.dma_start(wq_dst[:, :, C + P:2 * C],
                          wqkv_src[:, :, C + P:2 * C])
        wp_src = wproj_d.ap().rearrange("(k p) c -> p k c", p=P)
        wp_dst = wp_all[:].rearrange("p (k c) -> p k c", c=C)
        nc.sync.dma_start(wp_dst[:], wp_src[:])

        # PE warm-up burst (HAM clock gate) while the input DMAs land.
        with tc.tile_pool(name="warmp", bufs=1) as warmp, \
             tc.tile_pool(name="psw", bufs=1, space="PSUM") as psw:
            warm_sb = warmp.tile([P, 512], BF16, tag="warm", name="warm")
            nc.vector.memset(warm_sb[:], 0.0)
            warm_ps = psw.tile([P, 512], F32, tag="warmps", name="warmps")
            for _ in range(14):
                mm(warm_ps[:], warm_sb[:, 0:P], warm_sb[:],
                   start=True, stop=True)
            for (b0, bw) in ((0, 512), (512, 256)):
                bps = psw.tile([P, 512], F32, tag="bps", name="bps")
                mm(bps[:, 0:bw], ones[0:1, :], bproj_sb[0:1, b0:b0 + bw],
                   start=True, stop=True)
                nc.vector.tensor_copy(bias_bc[:, b0:b0 + bw], bps[:, 0:bw])

        et_tiles = {}

        with tc.tile_pool(name="ps_qk", bufs=2, space="PSUM") as ps_qk, \
             tc.tile_pool(name="ps_s", bufs=2, space="PSUM") as ps_s, \
             tc.tile_pool(name="ps_pv", bufs=2, space="PSUM") as ps_pv:

            def qkproj(hp):
                for m in (hp, NHALF + hp):
                    for n2 in range(NT2):
                        ps = ps_qk.tile([P, 512], F32, tag="qk",
                                        name=f"qkp{m}_{n2}")
                        for k in range(KC):
                            mm(ps[:], wq(k)[:, m * P:(m + 1) * P],
                               xT(k)[:, n2 * 512:(n2 + 1) * 512],
                               start=(k == 0), stop=(k == KC - 1))
                        nc.vector.tensor_copy(
                            qkT[m][:, n2 * 512:(n2 + 1) * 512], ps[:])

            def mkloop(hp):
                qt, kt = qkT[hp], qkT[NHALF + hp]
                with tc.high_priority(2500):
                    for mk in range(NT):
                        for half in range(2):
                            hr = slice(half * HD, (half + 1) * HD)
                            s = ps_s.tile([P, N], F32, tag="s",
                                          name=f"s{hp}_{mk}_{half}")
                            tkc = slice(mk * P, (mk + 1) * P)
                            for n2 in range(NT2):
                                mm(s[:, n2 * 512:(n2 + 1) * 512],
                                   kt[hr, tkc],
                                   qt[hr, n2 * 512:(n2 + 1) * 512],
                                   start=True, stop=True)
                            et = etp.tile([P, N], BF16, tag="et",
                                          name=f"et{hp}_{mk}_{half}")
                            nc.scalar.activation(et[:], s[:], EXP,
                                                 scale=SCALE)
                            et_tiles[(hp, half, mk)] = et

            def pv_norm(hp):
                for n2 in range(NT2):
                    # interleave the two heads' accumulation chains
                    # MM-by-MM: adjacent matmuls target different PSUM
                    # banks, maximizing weight-load pipelining
                    pvs = [ps_pv.tile([VA, 512], F32, tag="pv",
                                      name=f"pv{hp}_{half}_{n2}")
                           for half in range(2)]
                    for mk in range(NT):
                        for half in range(2):
                            h = 2 * hp + half
                            mm(pvs[half][:],
                               vaug[mk][:, h * VP:h * VP + VA],
                               et_tiles[(hp, half, mk)][
                                   :, n2 * 512:(n2 + 1) * 512],
                               start=(mk == 0), stop=(mk == NT - 1))
                    for half in range(2):
                        h = 2 * hp + half
                        hr = slice(half * HD, (half + 1) * HD)
                        pv = pvs[half]
                        sums = small.tile([1, 512], BF16, tag="sums",
                                          name="sums")
                        nc.vector.tensor_copy(sums[:], pv[HD:VA, :])
                        bc = ps_qk.tile([HD, 512], F32, tag="qk", name="bc")
                        mm(bc[:], ones[0:1, 0:HD], sums[:],
                           start=True, stop=True)
                        bc_sb = small.tile([HD, 512], F32, tag="bcs",
                                           name="bcs")
                        nc.vector.reciprocal_approx_fast(bc_sb[:], bc[:])
                        nc.vector.tensor_mul(
                            oT[hp][hr, n2 * 512:(n2 + 1) * 512],
                            pv[0:HD, :], bc_sb[:])

            qkproj(0)
            mkloop(0)
            qkproj(1)

            # v: token-major [tokens 128, feat] into v_aug tiles
            # (per-head stride VP=128: cols 0:64 data, 64 ones, rest pad)
            for t in range(NT):
                nc.gpsimd.memset(vaug[t][:], 0.0)
                for (n0, nw) in ((0, 512), (512, 256)):
                    ps = ps_qk.tile([P, 512], F32, tag="qk",
                                    name=f"v{t}_{n0}")
                    for k in range(KC):
                        mm(ps[:, 0:nw], xT(k)[:, t * P:(t + 1) * P],
                           wq(k)[:, 2 * C + n0: 2 * C + n0 + nw],
                           start=(k == 0), stop=(k == KC - 1))
                    h0, hn = n0 // HD, nw // HD
                    dst = vaug[t][:].rearrange("p (h m) -> p h m", m=VP)
                    src = ps[:, 0:nw].rearrange("p (h m) -> p h m", m=HD)
                    nc.vector.tensor_copy(dst[:, h0:h0 + hn, 0:HD], src)
                nc.vector.tensor_copy(
                    vaug[t][:].rearrange("p (h m) -> p h m",
                                         m=VP)[:, :, HD:HD + 1],
                    onesc[:].rearrange("p (h o) -> p h o", o=1))

            pv_norm(0)
            for hp in range(1, NHALF):
                mkloop(hp)
                if hp + 1 < NHALF:
                    qkproj(hp + 1)
                pv_norm(hp)

        # final projection: out[tq,:] = oT[:, tq].T @ w_proj + bias
        with tc.tile_pool(name="outp", bufs=3) as out_pool, \
             tc.tile_pool(name="ps3a", bufs=2, space="PSUM") as ps3a, \
             tc.tile_pool(name="ps3b", bufs=2, space="PSUM") as ps3b:
            for t in range(NT):
                tq = slice(t * P, (t + 1) * P)
                psa = ps3a.tile([P, 512], F32, tag="fa", name="fa")
                psb = ps3b.tile([P, 256], F32, tag="fb", name="fb")
                for k in range(KC):
                    mm(psa[:], oT[k][:, tq], wp(k)[:, 0:512],
                       start=(k == 0), stop=(k == KC - 1))
                    mm(psb[:], oT[k][:, tq], wp(k)[:, 512:768],
                       start=(k == 0), stop=(k == KC - 1))
                ot = out_pool.tile([P, C], F32, tag="out", name="outt")
                nc.vector.tensor_add(ot[:, 0:512], psa[:], bias_bc[:, 0:512])
                nc.sync.dma_start(out_d.ap()[tq, 0:512], ot[:, 0:512])
                nc.vector.tensor_add(ot[:, 512:768], psb[:],
                                     bias_bc[:, 512:768])
                nc.gpsimd.dma_start(out_d.ap()[tq, 512:768],
                                    ot[:, 512:768])


def build():
    if "nc" in _CACHE:
        return _CACHE["nc"]
    nc = bacc.Bacc("TRN2", target_bir_lowering=False, debug=False)
    with tile.TileContext(nc) as tc:
        _emit(nc, tc)
    nc.compile()
    _CACHE["nc"] = nc
    return nc


def make_in_maps(x, w_qkv, w_proj, b_proj):
    x = np.asarray(x, dtype=np.float32)
    w_qkv = np.asarray(w_qkv, dtype=np.float32).astype(ml_dtypes.bfloat16)
    w_proj = np.asarray(w_proj, dtype=np.float32).astype(ml_dtypes.bfloat16)
    b_proj = np.ascontiguousarray(
        np.asarray(b_proj, dtype=np.float32).reshape(1, C))
    return [
        {
            "xT": np.ascontiguousarray(x[i].T.astype(ml_dtypes.bfloat16)),
            "w_qkv": w_qkv,
            "w_proj": w_proj,
            "b_proj": b_proj.astype(ml_dtypes.bfloat16),
        }
        for i in range(B)
    ]


def run(x, w_qkv, w_proj, b_proj, **spmd_kwargs):
    nc = build()
    in_maps = make_in_maps(x, w_qkv, w_proj, b_proj)
    res = run_bass_kernel_spmd(nc, in_maps, core_ids=list(range(B)),
                               **spmd_kwargs)
    out = np.stack([res.results[i]["out"] for i in range(B)])
    return out.astype(np.float32), res


def kernel(x, w_qkv, w_proj, b_proj, H=None, W=None, **_ignored):
    out, _ = run(x, w_qkv, w_proj, b_proj)
    return out
